# revision 7
# baseline (speedup 1.0000x reference)
"""Trainium2 Bass kernel for multi-head attention (B=4, T=2048, D=1024, H=16, DH=64).

Sharding: 8 cores = data-parallel over B (4) x tensor-parallel over heads (2 groups
of 8 heads).  Core c handles batch c//2, head group c%2.  Wq/Wk/Wv are sharded
column-wise by head, Wo row-wise; the two partial output projections per batch are
summed on the host (cheaper than an on-device all-reduce given full-I/O contract).

Kernel orientation (transpose-free):
  - host supplies x^T (D,T) per core; weights in natural layout
  - Q^T,K^T = W^T x^T via PE (stationary = W tiles)
  - V computed in (T, inner) layout, augmented with a ones column per head
  - S^T = K Q^T per head; two heads packed in PE row groups (contraction=64 each)
  - P^T = exp(SCALE * S^T) on ACT directly PSUM->SBUF (no max subtraction:
    |scores| <= ~4 for this problem's distribution, exp is safe in fp32)
  - O^T(+denom) accumulated via stationary [V_h | 1] tiles; row 64 = softmax denom
  - normalize via reciprocal_approx_fast + DMA partition-broadcast + DVE multiply
  - Y^T = Wo^T O_norm^T; host transposes back and sums the TP pair + bias
"""

import sys

sys.path.insert(0, "/opt/trn_rl_repo")

import numpy as np
import ml_dtypes

B, T, D = 4, 2048, 1024
H, DH = 16, 64
INNER = H * DH
SCALE = DH ** -0.5
TPG = 2                  # tensor-parallel groups
N_CORES = 8
HL = H // TPG            # heads per core
IL = HL * DH             # inner-local width

_CACHE: dict = {}


def _build(t_len: int):
    import concourse.bass as bass
    import concourse.mybir as mybir
    import concourse.tile as tile
    from concourse import bacc

    f32 = mybir.dt.float32
    bf16 = mybir.dt.bfloat16
    EXP = mybir.ActivationFunctionType.Exp
    COPY = mybir.ActivationFunctionType.Copy

    KD = D // 128        # contraction tiles over D
    MI = IL // 128       # inner-local partition tiles (= head pairs)
    NQ = t_len // 512    # 512-wide tiles over T
    KT = t_len // 128    # 128-wide tiles over T
    MD = D // 128        # output-D partition tiles
    KI = IL // 128       # contraction tiles over inner-local

    nc = bacc.Bacc("TRN2", target_bir_lowering=False, debug=False)
    xT = nc.dram_tensor("xT", [D, t_len], bf16, kind="ExternalInput").ap()
    wq = nc.dram_tensor("wq", [D, IL], bf16, kind="ExternalInput").ap()
    wk = nc.dram_tensor("wk", [D, IL], bf16, kind="ExternalInput").ap()
    wv = nc.dram_tensor("wv", [D, IL], bf16, kind="ExternalInput").ap()
    wo = nc.dram_tensor("wo", [IL, D], bf16, kind="ExternalInput").ap()
    yT = nc.dram_tensor("yT", [D, t_len], f32, kind="ExternalOutput").ap()

    def bcast(ap, n):
        return bass.AP(tensor=ap.tensor, offset=ap.offset,
                       ap=[[0, n]] + [list(d) for d in ap.ap[1:]])

    with tile.TileContext(nc) as tc:
        import contextlib
        with contextlib.ExitStack() as ctx:
            persist = ctx.enter_context(tc.tile_pool(name="persist", bufs=1))

            xt_sb = [persist.tile([128, t_len], bf16, name=f"xt{i}", tag=f"xt{i}")
                     for i in range(KD)]
            wq_sb = [persist.tile([128, IL], bf16, name=f"wq{i}", tag=f"wq{i}")
                     for i in range(KD)]
            wk_sb = [persist.tile([128, IL], bf16, name=f"wk{i}", tag=f"wk{i}")
                     for i in range(KD)]
            wv_sb = [persist.tile([128, IL], bf16, name=f"wv{i}", tag=f"wv{i}")
                     for i in range(KD)]
            wo_sb = [persist.tile([128, D], bf16, name=f"wo{i}", tag=f"wo{i}")
                     for i in range(KI)]
            qt_sb = [persist.tile([128, t_len], bf16, name=f"qt{i}", tag=f"qt{i}")
                     for i in range(MI)]
            kt_sb = [persist.tile([128, t_len], bf16, name=f"kt{i}", tag=f"kt{i}")
                     for i in range(MI)]
            va_sb = [persist.tile([128, HL * 65], bf16, name=f"va{i}", tag=f"va{i}")
                     for i in range(KT)]
            on_sb = [persist.tile([128, t_len], bf16, name=f"on{i}", tag=f"on{i}")
                     for i in range(KI)]

            # load order = first-use order: wv+x (V proj), then per-m Q/K weights
            for i in range(KD):
                nc.sync.dma_start(out=wv_sb[i], in_=wv[i * 128:(i + 1) * 128, :])
                nc.sync.dma_start(out=xt_sb[i], in_=xT[i * 128:(i + 1) * 128, :])
            for i in range(KD):
                nc.sync.dma_start(out=wq_sb[i], in_=wq[i * 128:(i + 1) * 128, :])
                nc.sync.dma_start(out=wk_sb[i], in_=wk[i * 128:(i + 1) * 128, :])
            for i in range(KI):
                nc.sync.dma_start(out=wo_sb[i], in_=wo[i * 128:(i + 1) * 128, :])

            # ones columns of the augmented-V tiles (written once, before the
            # V copies which only touch the 64-wide head slices)
            for t in range(KT):
                nc.vector.memset(
                    va_sb[t].rearrange("p (h c) -> p h c", c=65)[:, :, 64:65], 1.0)
            ones64 = persist.tile([1, 64], bf16, name="ones64", tag="ones64")
            nc.vector.memset(ones64, 1.0)

            # One global PSUM layout (8 banks) so every phase can overlap:
            #   pj: 2x (128,512)  = 2 banks (projections + output projection)
            #   s:  2x (128,1024) = 4 banks (scores head-pair + denom bcast)
            #   o:  2x (65,512)   = 2 banks (attention-output accumulators)
            pj = ctx.enter_context(tc.tile_pool(name="pj", bufs=2, space="PSUM"))
            spool = ctx.enter_context(tc.tile_pool(name="spool", bufs=2,
                                                   space="PSUM"))
            opool = ctx.enter_context(tc.tile_pool(name="opool", bufs=2,
                                                   space="PSUM"))
            ppool = ctx.enter_context(tc.tile_pool(name="ppool", bufs=8))
            rpool = ctx.enter_context(tc.tile_pool(name="rpool", bufs=4))
            ystage = ctx.enter_context(tc.tile_pool(name="ystage", bufs=4))

            # ---- V in (T, inner) layout ---------------------------------------
            for t in range(KT):
                vps = pj.tile([128, IL], f32, name=f"vps{t}", tag="pj")
                for k in range(KD):
                    nc.tensor.matmul(
                        vps, lhsT=xt_sb[k][:, t * 128:(t + 1) * 128],
                        rhs=wv_sb[k], start=(k == 0), stop=(k == KD - 1))
                nc.vector.tensor_copy(
                    va_sb[t].rearrange("p (h c) -> p h c", c=65)[:, :, 0:64],
                    vps.rearrange("p (h c) -> p h c", c=64))

            # ---- Q^T / K^T projections, m-interleaved -------------------------
            for m in range(MI):
                for w_sb, dst in ((wq_sb, qt_sb), (wk_sb, kt_sb)):
                    for n in range(NQ):
                        acc = pj.tile([128, 512], f32, name=f"pj{m}{n}", tag="pj")
                        for k in range(KD):
                            nc.tensor.matmul(
                                acc,
                                lhsT=w_sb[k][:, m * 128:(m + 1) * 128],
                                rhs=xt_sb[k][:, n * 512:(n + 1) * 512],
                                start=(k == 0), stop=(k == KD - 1))
                        nc.vector.tensor_copy(
                            dst[m][:, n * 512:(n + 1) * 512], acc)

            # ---- attention (hp outer so it starts once qt[hp]/kt[hp] ready) ---
            def yproj(n):
                for m in range(MD):
                    acc = pj.tile([128, 512], f32, name=f"y{m}{n}", tag="pj")
                    for k in range(KI):
                        nc.tensor.matmul(
                            acc, lhsT=wo_sb[k][:, m * 128:(m + 1) * 128],
                            rhs=on_sb[k][:, n * 512:(n + 1) * 512],
                            start=(k == 0), stop=(k == KI - 1))
                    ys = ystage.tile([128, 512], f32, name=f"ys{m}{n}", tag="ys")
                    nc.vector.tensor_copy(ys, acc)
                    nc.sync.dma_start(
                        out=yT[m * 128:(m + 1) * 128, n * 512:(n + 1) * 512],
                        in_=ys)

            for hp in range(MI):
                h0, h1 = 2 * hp, 2 * hp + 1
                for n in range(NQ):
                    o0 = opool.tile([65, 512], f32, name=f"o0_{hp}{n}", tag="o")
                    o1 = opool.tile([65, 512], f32, name=f"o1_{hp}{n}", tag="o")
                    p_tiles = {}

                    def s_mms(k):
                        s = spool.tile([128, 1024], f32, name=f"s{hp}{n}{k}",
                                       tag="s")
                        nc.tensor.matmul(
                            s[:, 0:512],
                            lhsT=kt_sb[hp][0:64, k * 128:(k + 1) * 128],
                            rhs=qt_sb[hp][0:64, n * 512:(n + 1) * 512],
                            start=True, stop=True)
                        nc.tensor.matmul(
                            s[:, 512:1024],
                            lhsT=kt_sb[hp][64:128, k * 128:(k + 1) * 128],
                            rhs=qt_sb[hp][64:128, n * 512:(n + 1) * 512],
                            start=True, stop=True)
                        p = ppool.tile([128, 1024], bf16, name=f"p{hp}{n}{k}",
                                       tag="p")
                        nc.scalar.activation(p, s, EXP, scale=SCALE)
                        p_tiles[k] = p

                    def av_mms(k):
                        p = p_tiles.pop(k)
                        nc.tensor.matmul(
                            o0, lhsT=va_sb[k][:, h0 * 65:h0 * 65 + 65],
                            rhs=p[:, 0:512],
                            start=(k == 0), stop=(k == KT - 1))
                        nc.tensor.matmul(
                            o1, lhsT=va_sb[k][:, h1 * 65:h1 * 65 + 65],
                            rhs=p[:, 512:1024],
                            start=(k == 0), stop=(k == KT - 1))

                    # software pipeline: emit S(k+1) before AV(k) so the PE
                    # queue never waits on the exp(k) ACT latency
                    s_mms(0)
                    for k in range(KT):
                        if k + 1 < KT:
                            s_mms(k + 1)
                        av_mms(k)

                    # denominators: pack both heads into one (128,512) bank via
                    # concurrent col-group K=1 matmuls, one reciprocal, 2 muls
                    d0 = rpool.tile([1, 512], bf16, name=f"d0_{hp}{n}", tag="d0")
                    d1 = rpool.tile([1, 512], bf16, name=f"d1_{hp}{n}", tag="d1")
                    nc.vector.tensor_copy(d0, o0[64:65, :])
                    nc.vector.tensor_copy(d1, o1[64:65, :])
                    db = spool.tile([128, 512], f32, name=f"db{hp}{n}", tag="s")
                    nc.tensor.matmul(db[0:64, :], lhsT=ones64, rhs=d0,
                                     start=True, stop=True)
                    nc.tensor.matmul(db[64:128, :], lhsT=ones64, rhs=d1,
                                     start=True, stop=True)
                    rb = rpool.tile([128, 512], f32, name=f"rb_{hp}{n}", tag="rb")
                    nc.vector.reciprocal_approx_fast(rb, db)
                    nc.vector.tensor_mul(
                        on_sb[hp][0:64, n * 512:(n + 1) * 512],
                        o0[0:64, :], rb[0:64, :])
                    nc.vector.tensor_mul(
                        on_sb[hp][64:128, n * 512:(n + 1) * 512],
                        o1[0:64, :], rb[64:128, :])

                    # interleave the output projection for q-tile n once the
                    # last head pair has produced it
                    if hp == MI - 1:
                        yproj(n)

    nc.compile()
    return nc


def _get_nc(t_len: int = T):
    key = ("nc", t_len)
    if key not in _CACHE:
        _CACHE[key] = _build(t_len)
    return _CACHE[key]


def _numpy_reference(x, attention_mask, Wq, Wk, Wv, Wo, bo):
    Bx, Tx, _ = x.shape
    out = np.zeros((Bx, Tx, INNER), np.float32)
    for b in range(Bx):
        q = (x[b] @ Wq).reshape(Tx, H, DH)
        k = (x[b] @ Wk).reshape(Tx, H, DH)
        v = (x[b] @ Wv).reshape(Tx, H, DH)
        for h in range(H):
            s = (q[:, h] @ k[:, h].T) * SCALE + attention_mask[b, 0]
            s = s - s.max(axis=-1, keepdims=True)
            p = np.exp(s)
            p /= p.sum(axis=-1, keepdims=True)
            out[b, :, h * DH:(h + 1) * DH] = p @ v[:, h]
    return out @ Wo + bo


def kernel(x, attention_mask, Wq, Wk, Wv, Wo, bo):
    x = np.ascontiguousarray(np.asarray(x, dtype=np.float32))
    attention_mask = np.asarray(attention_mask, dtype=np.float32)
    Wq = np.asarray(Wq, dtype=np.float32)
    Wk = np.asarray(Wk, dtype=np.float32)
    Wv = np.asarray(Wv, dtype=np.float32)
    Wo = np.asarray(Wo, dtype=np.float32)
    bo = np.asarray(bo, dtype=np.float32)

    if np.any(attention_mask):
        # off-spec input (spec fills the mask with zeros); fall back to exact host math
        return _numpy_reference(x, attention_mask, Wq, Wk, Wv, Wo, bo).astype(np.float32)

    res = run_device(x, Wq, Wk, Wv, Wo)
    out = np.empty((B, T, D), np.float32)
    for b in range(B):
        acc = res.results[TPG * b]["yT"] + res.results[TPG * b + 1]["yT"]
        out[b] = acc.T + bo
    return out


def run_device(x, Wq, Wk, Wv, Wo, **run_kwargs):
    from concourse import bass_utils

    bf = ml_dtypes.bfloat16
    nc = _get_nc(T)
    in_maps = []
    for c in range(N_CORES):
        b, g = c // TPG, c % TPG
        in_maps.append({
            "xT": np.ascontiguousarray(x[b].T).astype(bf),
            "wq": np.ascontiguousarray(Wq[:, g * IL:(g + 1) * IL]).astype(bf),
            "wk": np.ascontiguousarray(Wk[:, g * IL:(g + 1) * IL]).astype(bf),
            "wv": np.ascontiguousarray(Wv[:, g * IL:(g + 1) * IL]).astype(bf),
            "wo": np.ascontiguousarray(Wo[g * IL:(g + 1) * IL, :]).astype(bf),
        })
    return bass_utils.run_bass_kernel_spmd(
        nc, in_maps, core_ids=list(range(N_CORES)), **run_kwargs)


# revision 11
# speedup vs baseline: 1.0896x; 1.0896x over previous
"""Trainium2 Bass kernel for multi-head attention (B=4, T=2048, D=1024, H=16, DH=64).

Sharding: 8 cores = data-parallel over B (4) x tensor-parallel over heads (2 groups
of 8 heads).  Core c handles batch c//2, head group c%2.  Wq/Wk/Wv are sharded
column-wise by head, Wo row-wise; the two partial output projections per batch are
summed on the host (cheaper than an on-device all-reduce given full-I/O contract).

Kernel orientation (transpose-free):
  - host supplies x^T (D,T) per core; weights in natural layout
  - Q^T,K^T = W^T x^T via PE (stationary = W tiles)
  - V computed in (T, inner) layout, augmented with a ones column per head
  - S^T = K Q^T per head; two heads packed in PE row groups (contraction=64 each)
  - P^T = exp(SCALE * S^T) on ACT directly PSUM->SBUF (no max subtraction:
    |scores| <= ~4 for this problem's distribution, exp is safe in fp32)
  - O^T(+denom) accumulated via stationary [V_h | 1] tiles; row 64 = softmax denom
  - normalize via reciprocal_approx_fast + DMA partition-broadcast + DVE multiply
  - Y^T = Wo^T O_norm^T; host transposes back and sums the TP pair + bias
"""

import sys

sys.path.insert(0, "/opt/trn_rl_repo")

import numpy as np
import ml_dtypes

B, T, D = 4, 2048, 1024
H, DH = 16, 64
INNER = H * DH
SCALE = DH ** -0.5
TPG = 2                  # tensor-parallel groups
N_CORES = 8
HL = H // TPG            # heads per core
IL = HL * DH             # inner-local width

_CACHE: dict = {}


def _build(t_len: int):
    import concourse.bass as bass
    import concourse.mybir as mybir
    import concourse.tile as tile
    from concourse import bacc

    f32 = mybir.dt.float32
    bf16 = mybir.dt.bfloat16
    EXP = mybir.ActivationFunctionType.Exp
    COPY = mybir.ActivationFunctionType.Copy

    KD = D // 128        # contraction tiles over D
    MI = IL // 128       # inner-local partition tiles (= head pairs)
    NQ = t_len // 512    # 512-wide tiles over T
    KT = t_len // 128    # 128-wide tiles over T
    MD = D // 128        # output-D partition tiles
    KI = IL // 128       # contraction tiles over inner-local

    nc = bacc.Bacc("TRN2", target_bir_lowering=False, debug=False)
    xT = nc.dram_tensor("xT", [D, t_len], bf16, kind="ExternalInput").ap()
    wq = nc.dram_tensor("wq", [D, IL], bf16, kind="ExternalInput").ap()
    wk = nc.dram_tensor("wk", [D, IL], bf16, kind="ExternalInput").ap()
    wv = nc.dram_tensor("wv", [D, IL], bf16, kind="ExternalInput").ap()
    wo = nc.dram_tensor("wo", [IL, D], bf16, kind="ExternalInput").ap()
    yT = nc.dram_tensor("yT", [D, t_len], f32, kind="ExternalOutput").ap()

    def bcast(ap, n):
        return bass.AP(tensor=ap.tensor, offset=ap.offset,
                       ap=[[0, n]] + [list(d) for d in ap.ap[1:]])

    with tile.TileContext(nc) as tc:
        import contextlib
        with contextlib.ExitStack() as ctx:
            persist = ctx.enter_context(tc.tile_pool(name="persist", bufs=1))

            xt_sb = [persist.tile([128, t_len], bf16, name=f"xt{i}", tag=f"xt{i}")
                     for i in range(KD)]
            wq_sb = [persist.tile([128, IL], bf16, name=f"wq{i}", tag=f"wq{i}")
                     for i in range(KD)]
            wk_sb = [persist.tile([128, IL], bf16, name=f"wk{i}", tag=f"wk{i}")
                     for i in range(KD)]
            wv_sb = [persist.tile([128, IL], bf16, name=f"wv{i}", tag=f"wv{i}")
                     for i in range(KD)]
            wo_sb = [persist.tile([128, D], bf16, name=f"wo{i}", tag=f"wo{i}")
                     for i in range(KI)]
            qt_sb = [persist.tile([128, t_len], bf16, name=f"qt{i}", tag=f"qt{i}")
                     for i in range(MI)]
            kt_sb = [persist.tile([128, t_len], bf16, name=f"kt{i}", tag=f"kt{i}")
                     for i in range(MI)]
            va_sb = [persist.tile([128, HL * 65], bf16, name=f"va{i}", tag=f"va{i}")
                     for i in range(KT)]
            on_sb = [persist.tile([128, t_len], bf16, name=f"on{i}", tag=f"on{i}")
                     for i in range(KI)]

            # load order = first-use order: wv+x (V proj), then per-m Q/K weights
            for i in range(KD):
                nc.sync.dma_start(out=wv_sb[i], in_=wv[i * 128:(i + 1) * 128, :])
                nc.sync.dma_start(out=xt_sb[i], in_=xT[i * 128:(i + 1) * 128, :])
            for i in range(KD):
                nc.sync.dma_start(out=wq_sb[i], in_=wq[i * 128:(i + 1) * 128, :])
                nc.sync.dma_start(out=wk_sb[i], in_=wk[i * 128:(i + 1) * 128, :])
            for i in range(KI):
                nc.sync.dma_start(out=wo_sb[i], in_=wo[i * 128:(i + 1) * 128, :])

            # ones columns of the augmented-V tiles (written once, before the
            # V copies which only touch the 64-wide head slices)
            for t in range(KT):
                nc.vector.memset(
                    va_sb[t].rearrange("p (h c) -> p h c", c=65)[:, :, 64:65], 1.0)
            ones64 = persist.tile([1, 64], bf16, name="ones64", tag="ones64")
            nc.vector.memset(ones64, 1.0)

            # One global PSUM layout (8 banks) so every phase can overlap:
            #   pj: 2x (128,512)  = 2 banks (projections + output projection)
            #   s:  2x (128,1024) = 4 banks (scores head-pair + denom bcast)
            #   o:  2x (65,512)   = 2 banks (attention-output accumulators)
            pj = ctx.enter_context(tc.tile_pool(name="pj", bufs=2, space="PSUM"))
            spool = ctx.enter_context(tc.tile_pool(name="spool", bufs=2,
                                                   space="PSUM"))
            opool = ctx.enter_context(tc.tile_pool(name="opool", bufs=2,
                                                   space="PSUM"))
            ppool = ctx.enter_context(tc.tile_pool(name="ppool", bufs=8))
            rpool = ctx.enter_context(tc.tile_pool(name="rpool", bufs=4))
            ystage = ctx.enter_context(tc.tile_pool(name="ystage", bufs=4))

            # ---- V in (T, inner) layout ---------------------------------------
            for t in range(KT):
                vps = pj.tile([128, IL], f32, name=f"vps{t}", tag="pj")
                for k in range(KD):
                    nc.tensor.matmul(
                        vps, lhsT=xt_sb[k][:, t * 128:(t + 1) * 128],
                        rhs=wv_sb[k], start=(k == 0), stop=(k == KD - 1))
                nc.vector.tensor_copy(
                    va_sb[t].rearrange("p (h c) -> p h c", c=65)[:, :, 0:64],
                    vps.rearrange("p (h c) -> p h c", c=64))

            # ---- Q^T / K^T projection emitters --------------------------------
            def proj_gen(m):
                for w_sb, dst in ((wq_sb, qt_sb), (wk_sb, kt_sb)):
                    for n in range(NQ):
                        acc = pj.tile([128, 512], f32,
                                      name=f"pj{m}{n}{dst is kt_sb}", tag="pj")
                        for k in range(KD):
                            nc.tensor.matmul(
                                acc,
                                lhsT=w_sb[k][:, m * 128:(m + 1) * 128],
                                rhs=xt_sb[k][:, n * 512:(n + 1) * 512],
                                start=(k == 0), stop=(k == KD - 1))
                            yield
                        nc.vector.tensor_copy(
                            dst[m][:, n * 512:(n + 1) * 512], acc)
                        yield

            # head-pair 0's projections run up front; later head pairs stream
            # as PE filler inside the previous head pair's attention (their
            # matmuls have no ACT dependency, so they fill the exp-wait
            # bubbles of the ACT-paced attention pipeline)
            for _ in proj_gen(0):
                pass

            from collections import deque
            fillers = deque()

            def pump(k=1):
                for _ in range(k):
                    while fillers:
                        try:
                            next(fillers[0])
                            break
                        except StopIteration:
                            fillers.popleft()
                    else:
                        return

            # ---- attention (hp outer so it starts once qt[hp]/kt[hp] ready) ---
            def yproj(n):
                for m in range(MD):
                    acc = pj.tile([128, 512], f32, name=f"y{m}{n}", tag="pj")
                    for k in range(KI):
                        nc.tensor.matmul(
                            acc, lhsT=wo_sb[k][:, m * 128:(m + 1) * 128],
                            rhs=on_sb[k][:, n * 512:(n + 1) * 512],
                            start=(k == 0), stop=(k == KI - 1))
                    ys = ystage.tile([128, 512], f32, name=f"ys{m}{n}", tag="ys")
                    nc.vector.tensor_copy(ys, acc)
                    nc.sync.dma_start(
                        out=yT[m * 128:(m + 1) * 128, n * 512:(n + 1) * 512],
                        in_=ys)

            for hp in range(MI):
                h0, h1 = 2 * hp, 2 * hp + 1
                if hp + 1 < MI:
                    fillers.append(proj_gen(hp + 1))
                for n in range(NQ):
                    o0 = opool.tile([65, 512], f32, name=f"o0_{hp}{n}", tag="o")
                    o1 = opool.tile([65, 512], f32, name=f"o1_{hp}{n}", tag="o")
                    p_tiles = {}

                    def s_mms(k):
                        s = spool.tile([128, 1024], f32, name=f"s{hp}{n}{k}",
                                       tag="s")
                        nc.tensor.matmul(
                            s[:, 0:512],
                            lhsT=kt_sb[hp][0:64, k * 128:(k + 1) * 128],
                            rhs=qt_sb[hp][0:64, n * 512:(n + 1) * 512],
                            start=True, stop=True)
                        nc.tensor.matmul(
                            s[:, 512:1024],
                            lhsT=kt_sb[hp][64:128, k * 128:(k + 1) * 128],
                            rhs=qt_sb[hp][64:128, n * 512:(n + 1) * 512],
                            start=True, stop=True)
                        p = ppool.tile([128, 1024], bf16, name=f"p{hp}{n}{k}",
                                       tag="p")
                        nc.scalar.activation(p, s, EXP, scale=SCALE)
                        p_tiles[k] = p

                    def av_mms(k):
                        p = p_tiles.pop(k)
                        nc.tensor.matmul(
                            o0, lhsT=va_sb[k][:, h0 * 65:h0 * 65 + 65],
                            rhs=p[:, 0:512],
                            start=(k == 0), stop=(k == KT - 1))
                        nc.tensor.matmul(
                            o1, lhsT=va_sb[k][:, h1 * 65:h1 * 65 + 65],
                            rhs=p[:, 512:1024],
                            start=(k == 0), stop=(k == KT - 1))

                    # software pipeline: emit S(k+1) before AV(k) so the PE
                    # queue never waits on the exp(k) ACT latency; one filler
                    # projection matmul per iteration soaks up the remaining
                    # ACT-pacing bubble
                    s_mms(0)
                    for k in range(KT):
                        if k + 1 < KT:
                            s_mms(k + 1)
                        pump(1)
                        av_mms(k)

                    # denominators: pack both heads into one (128,512) bank via
                    # concurrent col-group K=1 matmuls, one reciprocal, 2 muls
                    d0 = rpool.tile([1, 512], bf16, name=f"d0_{hp}{n}", tag="d0")
                    d1 = rpool.tile([1, 512], bf16, name=f"d1_{hp}{n}", tag="d1")
                    nc.vector.tensor_copy(d0, o0[64:65, :])
                    nc.vector.tensor_copy(d1, o1[64:65, :])
                    db = spool.tile([128, 512], f32, name=f"db{hp}{n}", tag="s")
                    nc.tensor.matmul(db[0:64, :], lhsT=ones64, rhs=d0,
                                     start=True, stop=True)
                    nc.tensor.matmul(db[64:128, :], lhsT=ones64, rhs=d1,
                                     start=True, stop=True)
                    rb = rpool.tile([128, 512], f32, name=f"rb_{hp}{n}", tag="rb")
                    nc.vector.reciprocal_approx_fast(rb, db)
                    nc.vector.tensor_mul(
                        on_sb[hp][0:64, n * 512:(n + 1) * 512],
                        o0[0:64, :], rb[0:64, :])
                    nc.vector.tensor_mul(
                        on_sb[hp][64:128, n * 512:(n + 1) * 512],
                        o1[0:64, :], rb[64:128, :])

                    pump(3)
                    # interleave the output projection for q-tile n once the
                    # last head pair has produced it
                    if hp == MI - 1:
                        yproj(n)

    nc.compile()
    return nc


def _get_nc(t_len: int = T):
    key = ("nc", t_len)
    if key not in _CACHE:
        _CACHE[key] = _build(t_len)
    return _CACHE[key]


def _numpy_reference(x, attention_mask, Wq, Wk, Wv, Wo, bo):
    Bx, Tx, _ = x.shape
    out = np.zeros((Bx, Tx, INNER), np.float32)
    for b in range(Bx):
        q = (x[b] @ Wq).reshape(Tx, H, DH)
        k = (x[b] @ Wk).reshape(Tx, H, DH)
        v = (x[b] @ Wv).reshape(Tx, H, DH)
        for h in range(H):
            s = (q[:, h] @ k[:, h].T) * SCALE + attention_mask[b, 0]
            s = s - s.max(axis=-1, keepdims=True)
            p = np.exp(s)
            p /= p.sum(axis=-1, keepdims=True)
            out[b, :, h * DH:(h + 1) * DH] = p @ v[:, h]
    return out @ Wo + bo


def kernel(x, attention_mask, Wq, Wk, Wv, Wo, bo):
    x = np.ascontiguousarray(np.asarray(x, dtype=np.float32))
    attention_mask = np.asarray(attention_mask, dtype=np.float32)
    Wq = np.asarray(Wq, dtype=np.float32)
    Wk = np.asarray(Wk, dtype=np.float32)
    Wv = np.asarray(Wv, dtype=np.float32)
    Wo = np.asarray(Wo, dtype=np.float32)
    bo = np.asarray(bo, dtype=np.float32)

    if np.any(attention_mask):
        # off-spec input (spec fills the mask with zeros); fall back to exact host math
        return _numpy_reference(x, attention_mask, Wq, Wk, Wv, Wo, bo).astype(np.float32)

    res = run_device(x, Wq, Wk, Wv, Wo)
    out = np.empty((B, T, D), np.float32)
    for b in range(B):
        acc = res.results[TPG * b]["yT"] + res.results[TPG * b + 1]["yT"]
        out[b] = acc.T + bo
    return out


def run_device(x, Wq, Wk, Wv, Wo, **run_kwargs):
    from concourse import bass_utils

    bf = ml_dtypes.bfloat16
    nc = _get_nc(T)
    in_maps = []
    for c in range(N_CORES):
        b, g = c // TPG, c % TPG
        in_maps.append({
            "xT": np.ascontiguousarray(x[b].T).astype(bf),
            "wq": np.ascontiguousarray(Wq[:, g * IL:(g + 1) * IL]).astype(bf),
            "wk": np.ascontiguousarray(Wk[:, g * IL:(g + 1) * IL]).astype(bf),
            "wv": np.ascontiguousarray(Wv[:, g * IL:(g + 1) * IL]).astype(bf),
            "wo": np.ascontiguousarray(Wo[g * IL:(g + 1) * IL, :]).astype(bf),
        })
    return bass_utils.run_bass_kernel_spmd(
        nc, in_maps, core_ids=list(range(N_CORES)), **run_kwargs)
